# revision 27
# baseline (speedup 1.0000x reference)
"""Trainium2 Bass kernel for the AHGCSP GCN layer problem.

Computes, per batch element b (8 total, one per NeuronCore):
    F   = Dynamic_L[b] * W[b,:,:,0] + Geo * W[b,:,:,1] + KL * W[b,:,:,2]
    P   = softmax(F, axis=-1)
    G1  = P @ inputs[b]
    out = tanh(G1 @ Wd + bd)

Device formulation (everything transposed host-side, free for HW time):
  - The six N*N operands are int8-quantized host-side with a per-m-column
    scale family chosen so all three products share one scale s(m):
      aq_k = rint(a_k / s_ak),  wq_k = rint(w_k * s_ak / s),  s = max_k s_ak/127
    so  F[n,m] = s(m) * sum_k aq_k[m,n] * wq_k[m,n].  This halves HBM traffic
    vs bf16 (the kernel is DMA-bound) at ~1.26e-2 rel err.
  - Per m-tile (128 m-rows): three pair-DMAs load [DL|W0], [Geo|W1],
    [KLa|W2a|KLb|W2b]. Products are split across engines by measured cost
    (DVE int8 mul ~1.04 ns/col, GPSIMD ~2.0, DVE fp16 2x ~0.52): p0 on DVE,
    p1 on GPSIMD, and the first CF cols of p2 via an ACT-engine int8->fp16
    upcast (one contiguous Copy) feeding a DVE 2x fp16 mul; the rest of p2
    on GPSIMD. The k-sum runs on DVE at fp16 2x; ScalarE applies exp with
    the per-partition scale AP s(m).
  - G1T_aug[f',r] = sum_m Xaug[m,f'] * P^T[m,r] accumulated in PSUM, where
    Xaug = [inputs[b] | ones] so row 64 of G1T_aug is the softmax denominator.
  - Epilogue in halves: 1/denom = exp(-ln(d)) on ScalarE, partition-broadcast
    via K=1 matmul, normalize, Dense(Wd, fp16 matmuls), tanh(+bd).
  - host transposes out^T back.
"""

import numpy as np

import bass_rust
import concourse.bass as bass
import concourse.mybir as mybir
from concourse.tile import TileContext
from concourse.bass_utils import run_bass_kernel_spmd

B, N, F, UNITS = 8, 2048, 64, 64
P = 128            # partitions
MT = N // P        # m-tiles per core (16)
FA = F + 1         # augmented feature dim (ones column)
CW = 3 * N         # product columns per m-tile (DL|Geo|KL) = 6144
QW = 512           # PSUM bank width in fp32 elements

# Engine split tunables (cols per 2048-wide m-tile). Defaults are the
# TimelineSim-tuned optimum; the AD/KEQ/KSP/KAP variants all measured worse
# (DMA-engine contention or longer per-tile chains) and stay off.
import os as _os
PV = int(_os.environ.get("KPV", "0"))      # DVE-int8 share of p1 product cols
PZ = int(_os.environ.get("KPZ", "2048"))   # DVE-int8 share of p0 product cols
CF = int(_os.environ.get("KCF", "1600"))   # ACT-upcast -> DVE-2x cols of p2
AD = int(_os.environ.get("KAD", "0"))      # cols of add2 via SWDGE accum-add DMA
IN_BUFS = int(_os.environ.get("KIB", "4"))
WORK_BUFS = int(_os.environ.get("KWB", "4"))
EPI_Q = int(_os.environ.get("KEQ", "0"))   # 1: late-normalize epilogue (broken)
SPLIT_P1 = int(_os.environ.get("KSP", "0"))  # 1: halve Pool's p1 mul for chain depth
AP = int(_os.environ.get("KAP", "0"))     # tail cols of each fusion add on Pool

FP32 = mybir.dt.float32
FP16 = mybir.dt.float16
I8 = mybir.dt.int8


def _cap_sync_waits(nc, max_waits=1):
    """The walrus build in this toolchain rejects instructions carrying more
    than a couple of sync waits ("Too many sync wait commands"). Hoist excess
    waits onto freshly inserted same-engine drain instructions immediately
    preceding the offender — identical blocking semantics, legal encoding."""
    eng_map = {
        mybir.EngineType.PE: nc.tensor,
        mybir.EngineType.DVE: nc.vector,
        mybir.EngineType.Activation: nc.scalar,
        mybir.EngineType.Pool: nc.gpsimd,
        mybir.EngineType.SP: nc.sync,
    }

    def _steal_fresh_drain(eng):
        binst = eng.drain()
        dmi = binst.ins
        for bb2 in nc.main_func.blocks:
            l2 = bb2.instructions
            if l2 and l2[-1].name == dmi.name:
                l2.pop()
                return dmi
        raise RuntimeError("could not find freshly appended drain")

    for bb in nc.main_func.blocks:
        il = bb.instructions
        i = 0
        while i < len(il):
            inst = il[i]
            si = inst.sync_info
            if si is not None and len(si.on_wait) > max_waits:
                waits = list(si.on_wait)
                extra, keep = waits[:-max_waits], waits[-max_waits:]
                eng = eng_map[inst.engine]
                for j in range(0, len(extra), max_waits):
                    dmi = _steal_fresh_drain(eng)
                    dmi.sync_info = bass_rust.SyncInfo(
                        on_wait=extra[j : j + max_waits], on_update=[]
                    )
                    il.insert(i, dmi)
                    i += 1
                inst.sync_info = bass_rust.SyncInfo(
                    on_wait=keep, on_update=list(si.on_update)
                )
            i += 1


def build_nc(passes: int = 1, in_bufs: int | None = None, work_bufs: int | None = None):
    """Build the per-core Bass graph. `passes` repeats the whole computation
    (for slope-based wall-clock timing); output is identical each pass."""
    if in_bufs is None:
        in_bufs = IN_BUFS
    if work_bufs is None:
        work_bufs = WORK_BUFS
    nc = bass.Bass(num_devices=B)

    awq = nc.declare_dram_parameter("awq", [P, MT * 2 * CW], I8, isOutput=False)
    scl = nc.declare_dram_parameter("scl", [P, MT], FP32, isOutput=False)
    xperm = nc.declare_dram_parameter("xperm", [P, MT * FA], FP16, isOutput=False)
    wd = nc.declare_dram_parameter("wd", [F, UNITS], FP16, isOutput=False)
    bdt = nc.declare_dram_parameter("bdt", [UNITS, 1], FP32, isOutput=False)
    outT = nc.declare_dram_parameter("outT", [UNITS, N], FP32, isOutput=True)

    with TileContext(nc) as tc:
        with (
            tc.tile_pool(name="consts", bufs=1) as cpool,
            tc.tile_pool(name="ins", bufs=in_bufs) as ipool,
            tc.tile_pool(name="work", bufs=work_bufs) as wpool,
            tc.tile_pool(name="epi", bufs=1) as epool,
            tc.tile_pool(name="psum", bufs=1, space="PSUM") as ppool,
        ):
            # packed DRAM layout per m-tile: [DL|W0 | Geo|W1 | KL|W2], N cols
            # each stream; loaded as three pair-DMAs so each product can start
            # as soon as its own operands land.
            def pair_dma(mi, j, tag):
                t = ipool.tile([P, 2 * N], I8, tag=tag)
                off = 2 * CW * mi + 2 * N * j
                nc.sync.dma_start(out=t[:, :], in_=awq[:, off : off + 2 * N])
                return t

            # prefetch tiles 0-1 before the bulk consts so compute starts
            # early; scl comes between (needed by tile 0's exp), x/wd/bd are
            # not needed until the first matmul / epilogue.
            pre = {(0, j): pair_dma(0, j, t) for j, t in
                   ((0, "pa"), (1, "pb"), (2, "pc"))}
            scl_sb = cpool.tile([P, MT], FP32, tag="scl")
            nc.sync.dma_start(out=scl_sb[:, :], in_=scl[:, :])
            pre.update({(1, j): pair_dma(1, j, t) for j, t in
                        ((0, "pa"), (1, "pb"), (2, "pc"))})

            x_sbuf = cpool.tile([P, MT * FA], FP16, tag="x")
            nc.sync.dma_start(out=x_sbuf[:, :], in_=xperm[:, :])
            wd_sbuf = cpool.tile([F, UNITS], FP16, tag="wd")
            nc.sync.dma_start(out=wd_sbuf[:, :], in_=wd[:, :])
            bd_sbuf = cpool.tile([UNITS, 1], FP32, tag="bd")
            nc.sync.dma_start(out=bd_sbuf[:, :], in_=bdt[:, :])
            ones_sb = cpool.tile([1, UNITS], FP16, tag="ones")
            nc.vector.memset(ones_sb[:, :], 1.0)

            for pi in range(passes):
                psum_g1 = ppool.tile([FA, N], FP32, tag="g1")
                for mi in range(MT):
                    if pi == 0 and mi in (0, 1):
                        pa, pb, pc = (pre[(mi, j)] for j in range(3))
                    else:
                        pa = pair_dma(mi, 0, "pa")
                        pb = pair_dma(mi, 1, "pb")
                        pc = pair_dma(mi, 2, "pc")
                    dl, w0 = pa[:, 0:N], pa[:, N : 2 * N]
                    geo, w1 = pb[:, 0:N], pb[:, N : 2 * N]
                    # pc layout: [KLa(CF) | W2a(CF) | KLb | W2b]
                    klb = pc[:, 2 * CF : N + CF]
                    w2b = pc[:, N + CF : 2 * N]

                    # products (int8 -> fp16), split across DVE / Pool / the
                    # ACT-upcast->DVE-2x route (fp16 operands run DVE at 2x)
                    p0 = wpool.tile([P, N], FP16, tag="p0")
                    if PZ > 0:
                        nc.vector.tensor_mul(p0[:, :PZ], dl[:, :PZ], w0[:, :PZ])
                    if PZ < N:
                        nc.gpsimd.tensor_mul(p0[:, PZ:], dl[:, PZ:], w0[:, PZ:])
                    p1 = wpool.tile([P, N], FP16, tag="p1")
                    if PV > 0:
                        nc.vector.tensor_mul(p1[:, :PV], geo[:, :PV], w1[:, :PV])
                    if SPLIT_P1:
                        mid = (PV + N) // 2
                        nc.gpsimd.tensor_mul(
                            p1[:, PV:mid], geo[:, PV:mid], w1[:, PV:mid]
                        )
                        nc.gpsimd.tensor_mul(
                            p1[:, mid:], geo[:, mid:], w1[:, mid:]
                        )
                    else:
                        nc.gpsimd.tensor_mul(p1[:, PV:], geo[:, PV:], w1[:, PV:])
                    p2 = wpool.tile([P, N], FP16, tag="p2")
                    if CF > 0:
                        t16 = wpool.tile([P, 2 * CF], FP16, tag="t16")
                        nc.scalar.activation(
                            t16[:, :],
                            pc[:, 0 : 2 * CF],
                            mybir.ActivationFunctionType.Copy,
                        )
                        nc.vector.tensor_mul(
                            p2[:, :CF], t16[:, :CF], t16[:, CF : 2 * CF]
                        )
                    if CF < N:
                        nc.gpsimd.tensor_mul(p2[:, CF:], klb[:, :], w2b[:, :])

                    # fusion sum: fs = p0 + p1 + p2 (DVE fp16 2x), optionally
                    # routing cols [0,AD) of the second add through a SWDGE
                    # accum-add DMA.
                    fs = wpool.tile([P, N], FP16, tag="fs")
                    if AD > 0:
                        nc.vector.tensor_add(fs[:, :AD], p0[:, :AD], p1[:, :AD])
                        nc.gpsimd.dma_start(
                            out=fs[:, :AD],
                            in_=p2[:, :AD],
                            accum_op=mybir.AluOpType.add,
                        )
                    if AD < N:
                        tp = wpool.tile([P, N - AD], FP16, tag="tp")
                        sp = N - AP  # cols [sp,N) of the adds run on Pool
                        if AP > 0:
                            nc.vector.tensor_add(
                                tp[:, : sp - AD], p0[:, AD:sp], p1[:, AD:sp]
                            )
                            nc.gpsimd.tensor_add(
                                tp[:, sp - AD :], p0[:, sp:], p1[:, sp:]
                            )
                            nc.vector.tensor_add(
                                fs[:, AD:sp], tp[:, : sp - AD], p2[:, AD:sp]
                            )
                            nc.gpsimd.tensor_add(
                                fs[:, sp:], tp[:, sp - AD :], p2[:, sp:]
                            )
                        else:
                            nc.vector.tensor_add(tp[:, :], p0[:, AD:], p1[:, AD:])
                            nc.vector.tensor_add(fs[:, AD:], tp[:, :], p2[:, AD:])

                    pt = wpool.tile([P, N], FP16, tag="pt")
                    nc.scalar.activation(
                        pt[:, :],
                        fs[:, :],
                        mybir.ActivationFunctionType.Exp,
                        scale=scl_sb[:, mi : mi + 1],
                    )

                    xa = x_sbuf[:, FA * mi : FA * (mi + 1)]
                    for q in range(N // QW):
                        nc.tensor.matmul(
                            psum_g1[:, QW * q : QW * (q + 1)],
                            xa,
                            pt[:, QW * q : QW * (q + 1)],
                            start=(mi == 0),
                            stop=(mi == MT - 1),
                        )

                if EPI_Q:
                    # late-normalize epilogue: h_raw = Wd^T @ numerator starts
                    # right after the copy (no wait on recip); ln/exp/bc of
                    # 1/denom overlap on ACT/PE; then one mul + tanh.
                    H = N // 2
                    for hh in range(2):
                        cs = slice(H * hh, H * (hh + 1))
                        g1t = epool.tile([F, H], FP16, tag=f"qg1t{hh}")
                        nc.vector.tensor_copy(g1t[:, :], psum_g1[:F, cs])
                        lnd = epool.tile([1, H], FP32, tag=f"qlnd{hh}")
                        nc.scalar.activation(
                            lnd[:, :],
                            psum_g1[F : F + 1, cs],
                            mybir.ActivationFunctionType.Ln,
                        )
                        recip = epool.tile([1, H], FP16, tag=f"qrec{hh}")
                        nc.scalar.activation(
                            recip[:, :],
                            lnd[:, :],
                            mybir.ActivationFunctionType.Exp,
                            scale=-1.0,
                        )
                        psum_h = ppool.tile([UNITS, H], FP32, tag="h")
                        psum_bc = ppool.tile([UNITS, H], FP32, tag="bc")
                        for q in range(2):
                            nc.tensor.matmul(
                                psum_h[:, QW * q : QW * (q + 1)],
                                wd_sbuf[:, :],
                                g1t[:, QW * q : QW * (q + 1)],
                                start=True, stop=True,
                            )
                            nc.tensor.matmul(
                                psum_bc[:, QW * q : QW * (q + 1)],
                                ones_sb[:, :UNITS],
                                recip[:, QW * q : QW * (q + 1)],
                                start=True, stop=True,
                            )
                        hn = epool.tile([UNITS, H], FP32, tag=f"qhn{hh}")
                        nc.vector.tensor_mul(hn[:, :], psum_h[:, :], psum_bc[:, :])
                        outt = epool.tile([UNITS, H], FP32, tag=f"qout{hh}")
                        nc.scalar.activation(
                            outt[:, :],
                            hn[:, :],
                            mybir.ActivationFunctionType.Tanh,
                            bias=bd_sbuf[:, :],
                        )
                        nc.sync.dma_start(out=outT[:, cs], in_=outt[:, :])
                    continue
                # epilogue in two r-halves. fp16 matmuls (1 PE pass instead of
                # 4) and ACT functions grouped across halves (Ln,Ln / Exp,Exp
                # / Tanh,Tanh) to minimize activation-table switches on HW.
                H = N // 2
                g1t_h, lnd_h, recip_h, g1n_h = [], [], [], []
                for hh in range(2):
                    cs = slice(H * hh, H * (hh + 1))
                    g1t = epool.tile([F, H], FP16, tag=f"g1t{hh}")
                    nc.vector.tensor_copy(g1t[:, :], psum_g1[:F, cs])
                    g1t_h.append(g1t)
                    lnd = epool.tile([1, H], FP32, tag=f"lnd{hh}")
                    nc.scalar.activation(
                        lnd[:, :],
                        psum_g1[F : F + 1, cs],
                        mybir.ActivationFunctionType.Ln,
                    )
                    lnd_h.append(lnd)
                for hh in range(2):
                    recip = epool.tile([1, H], FP16, tag=f"recip{hh}")
                    nc.scalar.activation(
                        recip[:, :],
                        lnd_h[hh][:, :],
                        mybir.ActivationFunctionType.Exp,
                        scale=-1.0,
                    )
                    recip_h.append(recip)
                for hh in range(2):
                    psum_bc = ppool.tile([F, H], FP32, tag="bc")
                    for q in range(2):
                        nc.tensor.matmul(
                            psum_bc[:, QW * q : QW * (q + 1)],
                            ones_sb[:, :F],
                            recip_h[hh][:, QW * q : QW * (q + 1)],
                            start=True,
                            stop=True,
                        )
                    g1n = epool.tile([F, H], FP16, tag=f"g1n{hh}")
                    nc.vector.tensor_mul(g1n[:, :], g1t_h[hh][:, :], psum_bc[:, :])
                    g1n_h.append(g1n)
                outt_h = []
                for hh in range(2):
                    psum_h = ppool.tile([UNITS, H], FP32, tag="h")
                    for q in range(2):
                        nc.tensor.matmul(
                            psum_h[:, QW * q : QW * (q + 1)],
                            wd_sbuf[:, :],
                            g1n_h[hh][:, QW * q : QW * (q + 1)],
                            start=True,
                            stop=True,
                        )
                    outt = epool.tile([UNITS, H], FP32, tag=f"outt{hh}")
                    nc.scalar.activation(
                        outt[:, :],
                        psum_h[:, :],
                        mybir.ActivationFunctionType.Tanh,
                        bias=bd_sbuf[:, :],
                    )
                    outt_h.append(outt)
                    cs = slice(H * hh, H * (hh + 1))
                    nc.sync.dma_start(out=outT[:, cs], in_=outt[:, :])

    _cap_sync_waits(nc)
    return nc


def prepare_in_maps(inputs, Dynamic_L, W, Geo, KL, Wd, bd):
    """Host-side sharding + layout/dtype transforms (not counted in HW time)."""
    inputs = np.ascontiguousarray(inputs, dtype=np.float32)
    Dynamic_L = np.asarray(Dynamic_L, dtype=np.float32)
    W = np.asarray(W, dtype=np.float32)
    Geo = np.asarray(Geo, dtype=np.float32)
    KL = np.asarray(KL, dtype=np.float32)
    wd = np.ascontiguousarray(np.asarray(Wd, dtype=np.float16))
    bdt = np.ascontiguousarray(np.asarray(bd, dtype=np.float32).reshape(UNITS, 1))

    # Shared (batch-independent) transposes/quantization for Geo, KL.
    GeoT = np.ascontiguousarray(Geo.T)  # [m, n]
    KLT = np.ascontiguousarray(KL.T)
    sGeo = np.maximum(np.max(np.abs(GeoT), axis=1), 1e-30) / 127.0  # [m]
    sKL = np.maximum(np.max(np.abs(KLT), axis=1), 1e-30) / 127.0
    aqGeo = np.rint(GeoT / sGeo[:, None]).astype(np.int8)
    aqKL = np.rint(KLT / sKL[:, None]).astype(np.int8)

    in_maps = []
    for b in range(B):
        DLT = Dynamic_L[b].T  # [m, n]
        sDL = np.maximum(np.max(np.abs(DLT), axis=1), 1e-30) / 127.0
        s = np.maximum(np.maximum(sDL, sGeo), sKL) / 127.0  # common product scale
        aqDL = np.rint(DLT / sDL[:, None]).astype(np.int8)
        wq0 = np.rint(W[b, :, :, 0].T * (sDL / s)[:, None]).astype(np.int8)
        wq1 = np.rint(W[b, :, :, 1].T * (sGeo / s)[:, None]).astype(np.int8)
        wq2 = np.rint(W[b, :, :, 2].T * (sKL / s)[:, None]).astype(np.int8)

        # Pack per m-tile: [DL | W0 | Geo | W1 | KLa | W2a | KLb | W2b],
        # 12288 cols. Operand pairs adjacent so each pair loads in one DMA;
        # KL/W2 split at CF so the ACT-upcast slice [KLa|W2a] is contiguous.
        def rs(x):
            return x.reshape(MT, P, N)

        kla, klb_ = rs(aqKL)[:, :, :CF], rs(aqKL)[:, :, CF:]
        w2a, w2b_ = rs(wq2)[:, :, :CF], rs(wq2)[:, :, CF:]
        blk = np.concatenate(
            [rs(aqDL), rs(wq0), rs(aqGeo), rs(wq1), kla, w2a, klb_, w2b_],
            axis=2,
        )  # [MT, P, 6*N]
        blk = blk.reshape(MT, P, 6, N)  # regroup for the transpose below
        awq_p = np.ascontiguousarray(
            blk.transpose(1, 0, 2, 3).reshape(P, MT * 2 * CW)
        )
        scl_p = np.ascontiguousarray(
            s.astype(np.float32).reshape(MT, P).T
        )  # [P, MT]

        xaug = np.concatenate(
            [inputs[b], np.ones((N, 1), dtype=np.float32)], axis=1
        )  # [N, FA]
        xperm = np.ascontiguousarray(
            xaug.reshape(MT, P, FA).transpose(1, 0, 2).reshape(P, MT * FA)
        ).astype(np.float16)

        in_maps.append(
            {
                "awq": awq_p,
                "scl": scl_p,
                "xperm": xperm,
                "wd": wd,
                "bdt": bdt,
            }
        )
    return in_maps


_NC_CACHE = {}


def _get_nc(passes=1):
    if passes not in _NC_CACHE:
        _NC_CACHE[passes] = build_nc(passes=passes)
    return _NC_CACHE[passes]


def kernel(**inputs) -> np.ndarray:
    in_maps = prepare_in_maps(**inputs)
    nc = _get_nc(passes=1)
    res = run_bass_kernel_spmd(nc, in_maps, core_ids=list(range(B)))
    out = np.stack([res.results[b]["outT"].T for b in range(B)], axis=0)
    return np.ascontiguousarray(out, dtype=np.float32)


if __name__ == "__main__":
    rng = np.random.default_rng(0)
    ins = {
        "inputs": rng.standard_normal((B, N, F), dtype=np.float32),
        "Dynamic_L": rng.standard_normal((B, N, N), dtype=np.float32),
        "W": rng.random((B, N, N, 3), dtype=np.float32),
        "Geo": rng.standard_normal((N, N), dtype=np.float32),
        "KL": rng.standard_normal((N, N), dtype=np.float32),
        "Wd": rng.standard_normal((F, UNITS), dtype=np.float32) / 8.0,
        "bd": np.zeros(UNITS, dtype=np.float32),
    }
    out = kernel(**ins)
    print("out", out.shape, out.dtype)



# revision 37
# speedup vs baseline: 2.4140x; 2.4140x over previous
"""Trainium2 Bass kernel for the AHGCSP GCN layer problem.

Computes, per batch element b (8 total, one per NeuronCore):
    F   = Dynamic_L[b] * W[b,:,:,0] + Geo * W[b,:,:,1] + KL * W[b,:,:,2]
    P   = softmax(F, axis=-1)
    G1  = P @ inputs[b]
    out = tanh(G1 @ Wd + bd)

Device formulation (everything transposed host-side, free for HW time):
  - The six N*N operands are int8-quantized host-side with a per-m-column
    scale family chosen so all three products share one scale s(m):
      aq_k = rint(a_k / s_ak),  wq_k = rint(w_k * s_ak / s),  s = max_k s_ak/127
    so  F[n,m] = s(m) * sum_k aq_k[m,n] * wq_k[m,n].  This halves HBM traffic
    vs bf16 (the kernel is DMA-bound) at ~1.26e-2 rel err.
  - Per m-tile (128 m-rows): four DMAs load fp16 [DLa|W0a] (the first MS
    cols of DL/W0 ship as fp16 *integers* — exact, and DVE multiplies fp16
    at 2x) plus int8 [DLb|W0b], [Geo|W1], [KLa|W2a|KLb|W2b]. Products are
    split across engines by measured cost (DVE int8 mul ~1.04 ns/col,
    GPSIMD ~2.0, DVE fp16 2x ~0.52): p0 on DVE (fp16+int8 slices), p1 on
    GPSIMD, the first CF cols of p2 via an ACT-engine int8->fp16 upcast
    (one contiguous Copy) feeding a DVE 2x mul, rest of p2 on GPSIMD. This
    balances DVE/Pool/ACT/DMA within ~5%. The k-sum runs on DVE at fp16 2x;
    ScalarE applies exp with the per-partition scale AP s(m).
  - G1T_aug[f',r] = sum_m Xaug[m,f'] * P^T[m,r] accumulated in PSUM, where
    Xaug = [inputs[b] | ones] so row 64 of G1T_aug is the softmax denominator.
  - Epilogue in halves: 1/denom = exp(-ln(d)) on ScalarE, partition-broadcast
    via K=1 matmul, normalize, Dense(Wd, fp16 matmuls), tanh(+bd).
  - host transposes out^T back.
"""

import numpy as np

import bass_rust
import concourse.bass as bass
import concourse.mybir as mybir
from concourse.tile import TileContext
from concourse.bass_utils import run_bass_kernel_spmd

B, N, F, UNITS = 8, 2048, 64, 64
P = 128            # partitions
MT = N // P        # m-tiles per core (16)
FA = F + 1         # augmented feature dim (ones column)
CW = 3 * N         # product columns per m-tile (DL|Geo|KL) = 6144
QW = 512           # PSUM bank width in fp32 elements

# Engine split tunables (cols per 2048-wide m-tile). Defaults are the
# TimelineSim-tuned optimum; the AD/KEQ/KSP/KAP variants all measured worse
# (DMA-engine contention or longer per-tile chains) and stay off.
import os as _os
PV = int(_os.environ.get("KPV", "0"))      # DVE-int8 share of p1 product cols
PZ = int(_os.environ.get("KPZ", "2048"))   # DVE-int8 share of p0 product cols
CF = int(_os.environ.get("KCF", "1600"))   # ACT-upcast -> DVE-2x cols of p2
AD = int(_os.environ.get("KAD", "0"))      # cols of add2 via SWDGE accum-add DMA
IN_BUFS = int(_os.environ.get("KIB", "3"))
WORK_BUFS = int(_os.environ.get("KWB", "4"))
EPI_Q = int(_os.environ.get("KEQ", "0"))   # 1: late-normalize epilogue (broken)
SPLIT_P1 = int(_os.environ.get("KSP", "0"))  # 1: halve Pool's p1 mul for chain depth
AP = int(_os.environ.get("KAP", "0"))     # tail cols of each fusion add on Pool
MS = int(_os.environ.get("KMS", "1152"))   # p0 cols shipped as fp16 ints (DVE 2x)

FP32 = mybir.dt.float32
FP16 = mybir.dt.float16
I8 = mybir.dt.int8


def _cap_sync_waits(nc, max_waits=1):
    """The walrus build in this toolchain rejects instructions carrying more
    than a couple of sync waits ("Too many sync wait commands"). Hoist excess
    waits onto freshly inserted same-engine drain instructions immediately
    preceding the offender — identical blocking semantics, legal encoding."""
    eng_map = {
        mybir.EngineType.PE: nc.tensor,
        mybir.EngineType.DVE: nc.vector,
        mybir.EngineType.Activation: nc.scalar,
        mybir.EngineType.Pool: nc.gpsimd,
        mybir.EngineType.SP: nc.sync,
    }

    def _steal_fresh_drain(eng):
        binst = eng.drain()
        dmi = binst.ins
        for bb2 in nc.main_func.blocks:
            l2 = bb2.instructions
            if l2 and l2[-1].name == dmi.name:
                l2.pop()
                return dmi
        raise RuntimeError("could not find freshly appended drain")

    for bb in nc.main_func.blocks:
        il = bb.instructions
        i = 0
        while i < len(il):
            inst = il[i]
            si = inst.sync_info
            if si is not None and len(si.on_wait) > max_waits:
                waits = list(si.on_wait)
                extra, keep = waits[:-max_waits], waits[-max_waits:]
                eng = eng_map[inst.engine]
                for j in range(0, len(extra), max_waits):
                    dmi = _steal_fresh_drain(eng)
                    dmi.sync_info = bass_rust.SyncInfo(
                        on_wait=extra[j : j + max_waits], on_update=[]
                    )
                    il.insert(i, dmi)
                    i += 1
                inst.sync_info = bass_rust.SyncInfo(
                    on_wait=keep, on_update=list(si.on_update)
                )
            i += 1


def build_nc(passes: int = 1, in_bufs: int | None = None, work_bufs: int | None = None):
    """Build the per-core Bass graph. `passes` repeats the whole computation
    (for slope-based wall-clock timing); output is identical each pass."""
    if in_bufs is None:
        in_bufs = IN_BUFS
    if work_bufs is None:
        work_bufs = WORK_BUFS
    nc = bass.Bass(num_devices=B)

    awq = nc.declare_dram_parameter(
        "awq", [P, MT * (2 * CW - 2 * MS)], I8, isOutput=False
    )
    awf = (
        nc.declare_dram_parameter("awf", [P, MT * 2 * MS], FP16, isOutput=False)
        if MS > 0
        else None
    )
    scl = nc.declare_dram_parameter("scl", [P, MT], FP32, isOutput=False)
    xperm = nc.declare_dram_parameter("xperm", [P, MT * FA], FP16, isOutput=False)
    wd = nc.declare_dram_parameter("wd", [F, UNITS], FP16, isOutput=False)
    bdt = nc.declare_dram_parameter("bdt", [UNITS, 1], FP32, isOutput=False)
    outT = nc.declare_dram_parameter("outT", [UNITS, N], FP16, isOutput=True)

    with TileContext(nc) as tc:
        with (
            tc.tile_pool(name="consts", bufs=1) as cpool,
            tc.tile_pool(name="ins", bufs=in_bufs) as ipool,
            tc.tile_pool(name="work", bufs=work_bufs) as wpool,
            tc.tile_pool(name="epi", bufs=1) as epool,
            tc.tile_pool(name="psum", bufs=1, space="PSUM") as ppool,
        ):
            # int8 DRAM layout per m-tile: [DLb|W0b | Geo|W1 | KLa|W2a|KLb|W2b]
            # (12288-2*MS cols); the fp16 [DLa|W0a] slice lives in awf. Four
            # DMAs per tile so each product starts as soon as its operands land.
            S8 = 2 * CW - 2 * MS  # int8 cols per tile
            PAW = 2 * (N - MS)    # int8 [DLb|W0b] width

            def pair_dma(mi, j, tag):
                if j == 3:  # fp16 [DLa|W0a]
                    t = ipool.tile([P, 2 * MS], FP16, tag=tag)
                    nc.sync.dma_start(
                        out=t[:, :], in_=awf[:, 2 * MS * mi : 2 * MS * (mi + 1)]
                    )
                    return t
                t = ipool.tile([P, PAW if j == 0 else 2 * N], I8, tag=tag)
                off = S8 * mi + (0 if j == 0 else PAW + 2 * N * (j - 1))
                nc.sync.dma_start(
                    out=t[:, :], in_=awq[:, off : off + t.shape[1]]
                )
                return t

            JT = ((0, "pa"), (1, "pb"), (2, "pc")) + (
                ((3, "pf"),) if MS > 0 else ()
            )
            # prefetch tiles 0-1 before the bulk consts so compute starts
            # early; scl comes between (needed by tile 0's exp), x/wd/bd are
            # not needed until the first matmul / epilogue.
            pre = {(0, j): pair_dma(0, j, t) for j, t in JT}
            scl_sb = cpool.tile([P, MT], FP32, tag="scl")
            nc.sync.dma_start(out=scl_sb[:, :], in_=scl[:, :])
            pre.update({(1, j): pair_dma(1, j, t) for j, t in JT})

            x_sbuf = cpool.tile([P, MT * FA], FP16, tag="x")
            nc.sync.dma_start(out=x_sbuf[:, :], in_=xperm[:, :])
            wd_sbuf = cpool.tile([F, UNITS], FP16, tag="wd")
            nc.sync.dma_start(out=wd_sbuf[:, :], in_=wd[:, :])
            bd_sbuf = cpool.tile([UNITS, 1], FP32, tag="bd")
            nc.sync.dma_start(out=bd_sbuf[:, :], in_=bdt[:, :])
            ones_sb = cpool.tile([1, UNITS], FP16, tag="ones")
            nc.vector.memset(ones_sb[:, :], 1.0)

            for pi in range(passes):
                psum_g1 = ppool.tile([FA, N], FP32, tag="g1")
                for mi in range(MT):
                    if pi == 0 and mi in (0, 1):
                        pa, pb, pc = (pre[(mi, j)] for j in range(3))
                        pf = pre[(mi, 3)] if MS > 0 else None
                    else:
                        pa = pair_dma(mi, 0, "pa")
                        pb = pair_dma(mi, 1, "pb")
                        pc = pair_dma(mi, 2, "pc")
                        pf = pair_dma(mi, 3, "pf") if MS > 0 else None
                    nm = N - MS
                    dl, w0 = pa[:, 0:nm], pa[:, nm : 2 * nm]
                    geo, w1 = pb[:, 0:N], pb[:, N : 2 * N]
                    # pc layout: [KLa(CF) | W2a(CF) | KLb | W2b]
                    klb = pc[:, 2 * CF : N + CF]
                    w2b = pc[:, N + CF : 2 * N]

                    # products (int8 -> fp16), split across DVE / Pool / the
                    # ACT-upcast->DVE-2x route (fp16 operands run DVE at 2x)
                    p0 = wpool.tile([P, N], FP16, tag="p0")
                    if MS > 0:
                        nc.vector.tensor_mul(
                            p0[:, :MS], pf[:, :MS], pf[:, MS : 2 * MS]
                        )
                    nc.vector.tensor_mul(p0[:, MS:], dl[:, :], w0[:, :])
                    p1 = wpool.tile([P, N], FP16, tag="p1")
                    if PV > 0:
                        nc.vector.tensor_mul(p1[:, :PV], geo[:, :PV], w1[:, :PV])
                    if SPLIT_P1:
                        mid = (PV + N) // 2
                        nc.gpsimd.tensor_mul(
                            p1[:, PV:mid], geo[:, PV:mid], w1[:, PV:mid]
                        )
                        nc.gpsimd.tensor_mul(
                            p1[:, mid:], geo[:, mid:], w1[:, mid:]
                        )
                    else:
                        nc.gpsimd.tensor_mul(p1[:, PV:], geo[:, PV:], w1[:, PV:])
                    p2 = wpool.tile([P, N], FP16, tag="p2")
                    if CF > 0:
                        t16 = wpool.tile([P, 2 * CF], FP16, tag="t16")
                        nc.scalar.activation(
                            t16[:, :],
                            pc[:, 0 : 2 * CF],
                            mybir.ActivationFunctionType.Copy,
                        )
                        nc.vector.tensor_mul(
                            p2[:, :CF], t16[:, :CF], t16[:, CF : 2 * CF]
                        )
                    if CF < N:
                        nc.gpsimd.tensor_mul(p2[:, CF:], klb[:, :], w2b[:, :])

                    # fusion sum: fs = p0 + p1 + p2 (DVE fp16 2x), optionally
                    # routing cols [0,AD) of the second add through a SWDGE
                    # accum-add DMA.
                    fs = wpool.tile([P, N], FP16, tag="fs")
                    if AD > 0:
                        nc.vector.tensor_add(fs[:, :AD], p0[:, :AD], p1[:, :AD])
                        nc.gpsimd.dma_start(
                            out=fs[:, :AD],
                            in_=p2[:, :AD],
                            accum_op=mybir.AluOpType.add,
                        )
                    if AD < N:
                        tp = wpool.tile([P, N - AD], FP16, tag="tp")
                        sp = N - AP  # cols [sp,N) of the adds run on Pool
                        if AP > 0:
                            nc.vector.tensor_add(
                                tp[:, : sp - AD], p0[:, AD:sp], p1[:, AD:sp]
                            )
                            nc.gpsimd.tensor_add(
                                tp[:, sp - AD :], p0[:, sp:], p1[:, sp:]
                            )
                            nc.vector.tensor_add(
                                fs[:, AD:sp], tp[:, : sp - AD], p2[:, AD:sp]
                            )
                            nc.gpsimd.tensor_add(
                                fs[:, sp:], tp[:, sp - AD :], p2[:, sp:]
                            )
                        else:
                            nc.vector.tensor_add(tp[:, :], p0[:, AD:], p1[:, AD:])
                            nc.vector.tensor_add(fs[:, AD:], tp[:, :], p2[:, AD:])

                    pt = wpool.tile([P, N], FP16, tag="pt")
                    nc.scalar.activation(
                        pt[:, :],
                        fs[:, :],
                        mybir.ActivationFunctionType.Exp,
                        scale=scl_sb[:, mi : mi + 1],
                    )

                    xa = x_sbuf[:, FA * mi : FA * (mi + 1)]
                    for q in range(N // QW):
                        nc.tensor.matmul(
                            psum_g1[:, QW * q : QW * (q + 1)],
                            xa,
                            pt[:, QW * q : QW * (q + 1)],
                            start=(mi == 0),
                            stop=(mi == MT - 1),
                        )

                if EPI_Q:
                    # late-normalize epilogue: h_raw = Wd^T @ numerator starts
                    # right after the copy (no wait on recip); ln/exp/bc of
                    # 1/denom overlap on ACT/PE; then one mul + tanh.
                    H = N // 2
                    for hh in range(2):
                        cs = slice(H * hh, H * (hh + 1))
                        g1t = epool.tile([F, H], FP16, tag=f"qg1t{hh}")
                        nc.vector.tensor_copy(g1t[:, :], psum_g1[:F, cs])
                        lnd = epool.tile([1, H], FP32, tag=f"qlnd{hh}")
                        nc.scalar.activation(
                            lnd[:, :],
                            psum_g1[F : F + 1, cs],
                            mybir.ActivationFunctionType.Ln,
                        )
                        recip = epool.tile([1, H], FP16, tag=f"qrec{hh}")
                        nc.scalar.activation(
                            recip[:, :],
                            lnd[:, :],
                            mybir.ActivationFunctionType.Exp,
                            scale=-1.0,
                        )
                        psum_h = ppool.tile([UNITS, H], FP32, tag="h")
                        psum_bc = ppool.tile([UNITS, H], FP32, tag="bc")
                        for q in range(2):
                            nc.tensor.matmul(
                                psum_h[:, QW * q : QW * (q + 1)],
                                wd_sbuf[:, :],
                                g1t[:, QW * q : QW * (q + 1)],
                                start=True, stop=True,
                            )
                            nc.tensor.matmul(
                                psum_bc[:, QW * q : QW * (q + 1)],
                                ones_sb[:, :UNITS],
                                recip[:, QW * q : QW * (q + 1)],
                                start=True, stop=True,
                            )
                        hn = epool.tile([UNITS, H], FP32, tag=f"qhn{hh}")
                        nc.vector.tensor_mul(hn[:, :], psum_h[:, :], psum_bc[:, :])
                        outt = epool.tile([UNITS, H], FP32, tag=f"qout{hh}")
                        nc.scalar.activation(
                            outt[:, :],
                            hn[:, :],
                            mybir.ActivationFunctionType.Tanh,
                            bias=bd_sbuf[:, :],
                        )
                        nc.sync.dma_start(out=outT[:, cs], in_=outt[:, :])
                    continue
                # epilogue in two r-halves. fp16 matmuls (1 PE pass instead of
                # 4) and ACT functions grouped across halves (Ln,Ln / Exp,Exp
                # / Tanh,Tanh) to minimize activation-table switches on HW.
                H = N // 2
                g1t_h, lnd_h, recip_h, g1n_h = [], [], [], []
                for hh in range(2):
                    cs = slice(H * hh, H * (hh + 1))
                    g1t = epool.tile([F, H], FP16, tag=f"g1t{hh}")
                    nc.vector.tensor_copy(g1t[:, :], psum_g1[:F, cs])
                    g1t_h.append(g1t)
                    lnd = epool.tile([1, H], FP32, tag=f"lnd{hh}")
                    nc.scalar.activation(
                        lnd[:, :],
                        psum_g1[F : F + 1, cs],
                        mybir.ActivationFunctionType.Ln,
                    )
                    lnd_h.append(lnd)
                for hh in range(2):
                    recip = epool.tile([1, H], FP16, tag=f"recip{hh}")
                    nc.scalar.activation(
                        recip[:, :],
                        lnd_h[hh][:, :],
                        mybir.ActivationFunctionType.Exp,
                        scale=-1.0,
                    )
                    recip_h.append(recip)
                for hh in range(2):
                    psum_bc = ppool.tile([F, H], FP32, tag="bc")
                    for q in range(2):
                        nc.tensor.matmul(
                            psum_bc[:, QW * q : QW * (q + 1)],
                            ones_sb[:, :F],
                            recip_h[hh][:, QW * q : QW * (q + 1)],
                            start=True,
                            stop=True,
                        )
                    g1n = epool.tile([F, H], FP16, tag=f"g1n{hh}")
                    nc.vector.tensor_mul(g1n[:, :], g1t_h[hh][:, :], psum_bc[:, :])
                    g1n_h.append(g1n)
                outt_h = []
                for hh in range(2):
                    psum_h = ppool.tile([UNITS, H], FP32, tag="h")
                    for q in range(2):
                        nc.tensor.matmul(
                            psum_h[:, QW * q : QW * (q + 1)],
                            wd_sbuf[:, :],
                            g1n_h[hh][:, QW * q : QW * (q + 1)],
                            start=True,
                            stop=True,
                        )
                    outt = epool.tile([UNITS, H], FP16, tag=f"outt{hh}")
                    nc.scalar.activation(
                        outt[:, :],
                        psum_h[:, :],
                        mybir.ActivationFunctionType.Tanh,
                        bias=bd_sbuf[:, :],
                    )
                    outt_h.append(outt)
                    cs = slice(H * hh, H * (hh + 1))
                    nc.sync.dma_start(out=outT[:, cs], in_=outt[:, :])

    _cap_sync_waits(nc)
    return nc


def prepare_in_maps(inputs, Dynamic_L, W, Geo, KL, Wd, bd):
    """Host-side sharding + layout/dtype transforms (not counted in HW time)."""
    inputs = np.ascontiguousarray(inputs, dtype=np.float32)
    Dynamic_L = np.asarray(Dynamic_L, dtype=np.float32)
    W = np.asarray(W, dtype=np.float32)
    Geo = np.asarray(Geo, dtype=np.float32)
    KL = np.asarray(KL, dtype=np.float32)
    wd = np.ascontiguousarray(np.asarray(Wd, dtype=np.float16))
    bdt = np.ascontiguousarray(np.asarray(bd, dtype=np.float32).reshape(UNITS, 1))

    # Shared (batch-independent) transposes/quantization for Geo, KL.
    GeoT = np.ascontiguousarray(Geo.T)  # [m, n]
    KLT = np.ascontiguousarray(KL.T)
    sGeo = np.maximum(np.max(np.abs(GeoT), axis=1), 1e-30) / 127.0  # [m]
    sKL = np.maximum(np.max(np.abs(KLT), axis=1), 1e-30) / 127.0
    aqGeo = np.rint(GeoT / sGeo[:, None]).astype(np.int8)
    aqKL = np.rint(KLT / sKL[:, None]).astype(np.int8)

    in_maps = []
    for b in range(B):
        DLT = Dynamic_L[b].T  # [m, n]
        sDL = np.maximum(np.max(np.abs(DLT), axis=1), 1e-30) / 127.0
        s = np.maximum(np.maximum(sDL, sGeo), sKL) / 127.0  # common product scale
        aqDL = np.rint(DLT / sDL[:, None]).astype(np.int8)
        wq0 = np.rint(W[b, :, :, 0].T * (sDL / s)[:, None]).astype(np.int8)
        wq1 = np.rint(W[b, :, :, 1].T * (sGeo / s)[:, None]).astype(np.int8)
        wq2 = np.rint(W[b, :, :, 2].T * (sKL / s)[:, None]).astype(np.int8)

        # Pack per m-tile. int8 block: [DLb | W0b | Geo | W1 | KLa | W2a |
        # KLb | W2b] (12288-2*MS cols). The first MS cols of DL/W0 ship as
        # fp16 *integers* (exact — quantized values are in [-127,127]) in a
        # separate awf stream so DVE multiplies them in 2x mode.
        # Operand pairs adjacent so each pair loads in one DMA; KL/W2 split
        # at CF so the ACT-upcast slice [KLa|W2a] is contiguous.
        def rs(x):
            return x.reshape(MT, P, N)

        dla, dlb = rs(aqDL)[:, :, :MS], rs(aqDL)[:, :, MS:]
        w0a, w0b = rs(wq0)[:, :, :MS], rs(wq0)[:, :, MS:]
        kla, klb_ = rs(aqKL)[:, :, :CF], rs(aqKL)[:, :, CF:]
        w2a, w2b_ = rs(wq2)[:, :, :CF], rs(wq2)[:, :, CF:]
        blk = np.concatenate(
            [dlb, w0b, rs(aqGeo), rs(wq1), kla, w2a, klb_, w2b_],
            axis=2,
        )  # [MT, P, 6*N - 2*MS]
        awq_p = np.ascontiguousarray(
            blk.transpose(1, 0, 2).reshape(P, MT * (2 * CW - 2 * MS))
        )
        awf_p = np.ascontiguousarray(
            np.concatenate([dla, w0a], axis=2)
            .astype(np.float16)
            .transpose(1, 0, 2)
            .reshape(P, MT * 2 * MS)
        )
        scl_p = np.ascontiguousarray(
            s.astype(np.float32).reshape(MT, P).T
        )  # [P, MT]

        xaug = np.concatenate(
            [inputs[b], np.ones((N, 1), dtype=np.float32)], axis=1
        )  # [N, FA]
        xperm = np.ascontiguousarray(
            xaug.reshape(MT, P, FA).transpose(1, 0, 2).reshape(P, MT * FA)
        ).astype(np.float16)

        in_maps.append(
            {
                "awq": awq_p,
                "awf": awf_p,
                "scl": scl_p,
                "xperm": xperm,
                "wd": wd,
                "bdt": bdt,
            }
        )
    return in_maps


_NC_CACHE = {}


def _get_nc(passes=1):
    if passes not in _NC_CACHE:
        _NC_CACHE[passes] = build_nc(passes=passes)
    return _NC_CACHE[passes]


def kernel(**inputs) -> np.ndarray:
    in_maps = prepare_in_maps(**inputs)
    nc = _get_nc(passes=1)
    res = run_bass_kernel_spmd(nc, in_maps, core_ids=list(range(B)))
    out = np.stack([res.results[b]["outT"].T for b in range(B)], axis=0)
    return np.ascontiguousarray(out, dtype=np.float32)


if __name__ == "__main__":
    rng = np.random.default_rng(0)
    ins = {
        "inputs": rng.standard_normal((B, N, F), dtype=np.float32),
        "Dynamic_L": rng.standard_normal((B, N, N), dtype=np.float32),
        "W": rng.random((B, N, N, 3), dtype=np.float32),
        "Geo": rng.standard_normal((N, N), dtype=np.float32),
        "KL": rng.standard_normal((N, N), dtype=np.float32),
        "Wd": rng.standard_normal((F, UNITS), dtype=np.float32) / 8.0,
        "bd": np.zeros(UNITS, dtype=np.float32),
    }
    out = kernel(**ins)
    print("out", out.shape, out.dtype)

